# revision 9
# baseline (speedup 1.0000x reference)
"""Trainium2 Bass kernel for nn_Classifier (segment mean-pool + tiny MLP head).

Pipeline (matches the jax reference):
  pooled[g] = mean of features over nodes with batch id g   (2048 graphs)
  out = LeakyReLU(LayerNorm(pooled @ W1 + b1)) @ W2 + b2    -> [2048, 1]

Design (v2, tuned from hardware microbenchmarks):
  * Data-parallel over 8 cores at 32-segment block granularity: core i owns
    graphs [256i, 256i+256) = 8 blocks of 32 segments, and exactly the nodes
    belonging to them (batch ids are sorted).
  * Features are quantized host-side to fp8 (e4m3) with error feedback along
    the node order: q_i = Q(x_i + c_{i-1}), c_i = x_i + c_{i-1} - q_i. Segment
    sums of q then telescope, so the quantization error per (segment, feature)
    sum is bounded by two carry terms (~0.04 std) instead of sqrt(n)*fp8
    noise. Halves HBM traffic vs bf16; DMA is the roofline.
  * Segment sums via one-hot matmul on the tensor engine in fp8 DoubleRow
    mode: each matmul contracts 256 nodes (2 k-tiles x 128 partitions)
    against a [128, 2, 32] one-hot, accumulating [32 segs, 256 feats] in
    PSUM per block. ~109 ns/matmul measured (issue-rate bound).
  * One-hots are built on the vector engine in batches of 8 groups with a
    single tensor_tensor is_equal (iota vs broadcast seg ids, bf16 in ->
    fp8 out). DVE measured ~1.2 ns/elem; 32-wide windows keep this off the
    critical path. GpSimd/Act are avoided (measured 2169/800 ns per tile).
  * No division by counts: the head computes h' = n_g * h and LayerNorm
    with a per-graph eps' = eps * max(n_g,1)^2, which is exactly equivalent
    (LN is scale-invariant up to the eps term). b1 enters as counts x b1
    via a K=1 matmul, so the head matches the reference for any b1.
  * The MLP head for segments [0,128) runs mid-stream once blocks 0-3 are
    done; the head for [128,256) runs at the end.
"""

from contextlib import ExitStack

import numpy as np
import ml_dtypes

import concourse.bass as bass
import concourse.mybir as mybir
import concourse.tile as tile
from concourse.bass_utils import run_bass_kernel_spmd

# ---------------------------------------------------------------------------
# Workaround: this walrus build rejects instructions carrying more than one
# semaphore wait ("Too many sync wait commands"), but Tile's semaphore
# assignment freely attaches several. After the TileContext has lowered the
# program, split any excess waits onto same-engine nops inserted right before
# the instruction (semantics are identical: all waits are monotonic and must
# hold before the instruction issues).
_MAX_WAITS = 1


def _split_excess_waits(nc: "bass.Bass", max_waits: int = _MAX_WAITS) -> None:
    ctr = 0
    for f in nc.m.functions:
        for b in f.blocks:
            out = []
            for inst in b.instructions:
                si = inst.sync_info
                waits = list(si.on_wait) if (si is not None and si.on_wait) else []
                if len(waits) > max_waits:
                    keep = waits[-max_waits:]
                    extra = waits[:-max_waits]
                    # On the PE queue the carrier must be a DRAIN: silicon
                    # promotes waitless LDWEIGHTS past in-flight work, so a
                    # plain nop's wait can be bypassed (walrus attaches a
                    # matmul's waits to its LDWEIGHTS — stripping them onto a
                    # nop re-opens that race). A drain fully serializes.
                    is_pe = inst.engine == mybir.EngineType.PE
                    for i in range(0, len(extra), max_waits):
                        ctr += 1
                        if is_pe:
                            nop = mybir.InstDrain(
                                name=f"waitsplit_drain_{ctr}", ins=[], outs=[],
                                engine=inst.engine,
                            )
                        else:
                            nop = mybir.InstNoOp(
                                name=f"waitsplit_nop_{ctr}", ins=[], outs=[],
                                engine=inst.engine,
                            )
                        nop.sync_info = mybir.SyncInfo(
                            on_wait=extra[i : i + max_waits], on_update=[]
                        )
                        nc.register_instruction(nop)
                        out.append(nop)
                    inst.sync_info = mybir.SyncInfo(
                        on_wait=keep, on_update=list(si.on_update or [])
                    )
                out.append(inst)
            b.instructions = out
# ---------------------------------------------------------------------------

N_CORES = 8
NUM_GRAPHS = 2048
SEGS_PER_CORE = NUM_GRAPHS // N_CORES  # 256
W = 32  # segment-block width (one PSUM accumulator per block)
NBLK = SEGS_PER_CORE // W  # 8 blocks per core
D = 256
K2 = 16  # 256-node groups per DMA chunk (chunk = 4096 nodes, 1 MB fp8)
LN_EPS = 1e-5
NEG_SLOPE = 0.01

_F32 = mybir.dt.float32
_BF16 = mybir.dt.bfloat16
_F8 = mybir.dt.float8e4
_ALU = mybir.AluOpType
_ACT = mybir.ActivationFunctionType
_FP8NP = ml_dtypes.float8_e4m3fn

# Test/debug hooks: set PROFILE=True before calling kernel() to request an
# NTFF trace; the BassKernelResults lands in LAST_RESULT.
PROFILE = False
PROFILE_DIR = None
LAST_RESULT = None

_PROGRAM_CACHE = {}


def _build_program(G: int) -> bass.Bass:
    """G = groups (of 256 nodes) per 32-segment block."""
    assert (8 * G) % K2 == 0
    NCH = 8 * G // K2
    nc = bass.Bass("TRN2", debug=False)
    feat_d = nc.dram_tensor("feat", [NCH, 128, K2 * 2 * 256], _F8, kind="ExternalInput").ap()
    segT_d = nc.dram_tensor("segT", [128, 16 * G], _BF16, kind="ExternalInput").ap()
    iota_d = nc.dram_tensor("iota", [128, 2 * K2, W], _BF16, kind="ExternalInput").ap()
    ident_d = nc.dram_tensor("ident", [32, 32], _F32, kind="ExternalInput").ap()
    id128_d = nc.dram_tensor("id128", [128, 128], _F32, kind="ExternalInput").ap()
    w1_d = nc.dram_tensor("w1", [128, 2, 128], _F32, kind="ExternalInput").ap()
    b1_d = nc.dram_tensor("b1row", [1, 128], _F32, kind="ExternalInput").ap()
    ct_d = nc.dram_tensor("countsT", [1, 256], _F32, kind="ExternalInput").ap()
    epsg_d = nc.dram_tensor("epsg", [128, 2], _F32, kind="ExternalInput").ap()
    pvec_d = nc.dram_tensor("pvec", [1, 385], _F32, kind="ExternalInput").ap()
    out_d = nc.dram_tensor("out", [2, 128], _F32, kind="ExternalOutput").ap()

    with tile.TileContext(nc) as tc, ExitStack() as ctx:
        cpool = ctx.enter_context(tc.tile_pool(name="consts", bufs=1))
        fpool = ctx.enter_context(tc.tile_pool(name="feat", bufs=6))
        opool = ctx.enter_context(tc.tile_pool(name="oh", bufs=8))
        acc = ctx.enter_context(tc.tile_pool(name="acc", bufs=1, space="PSUM"))
        ppool = ctx.enter_context(tc.tile_pool(name="pw", bufs=1, space="PSUM"))
        spool = ctx.enter_context(tc.tile_pool(name="small", bufs=2))

        iota_t = cpool.tile([128, 2 * K2, W], _BF16, tag="iota")
        nc.gpsimd.dma_start(out=iota_t[:], in_=iota_d[:])
        segT_t = cpool.tile([128, 16 * G], _BF16, tag="segT")
        nc.gpsimd.dma_start(out=segT_t[:], in_=segT_d[:])
        ident_t = cpool.tile([32, 32], _F32, tag="ident")
        nc.gpsimd.dma_start(out=ident_t[:], in_=ident_d[:])
        id128_t = cpool.tile([128, 128], _F32, tag="id128")
        nc.gpsimd.dma_start(out=id128_t[:], in_=id128_d[:])
        w1_t = cpool.tile([128, 2, 128], _F32, tag="w1")
        nc.gpsimd.dma_start(out=w1_t[:], in_=w1_d[:])
        b1_t = cpool.tile([1, 128], _F32, tag="b1")
        nc.gpsimd.dma_start(out=b1_t[:], in_=b1_d[:])
        ct_t = cpool.tile([1, 256], _F32, tag="ct")
        nc.gpsimd.dma_start(out=ct_t[:], in_=ct_d[:])
        epsg_t = cpool.tile([128, 2], _F32, tag="epsg")
        nc.gpsimd.dma_start(out=epsg_t[:], in_=epsg_d[:])
        pv_t = cpool.tile([1, 385], _F32, tag="pv")
        nc.gpsimd.dma_start(out=pv_t[:], in_=pvec_d[:])
        ones_row = cpool.tile([1, 128], _F32, tag="ones")
        nc.vector.memset(ones_row[:], 1.0)

        # broadcast [gamma | beta | W2 | b2] to all 128 partitions
        bc_ps = ppool.tile([128, 385], _F32, tag="bc")
        nc.tensor.matmul(
            out=bc_ps[:], lhsT=ones_row[:], rhs=pv_t[:], start=True, stop=True
        )
        bc = cpool.tile([128, 385], _F32, tag="bcs")
        nc.scalar.copy(bc[:], bc_ps[:])

        warm = cpool.tile([128, 2, 256], _F8, tag="warm")
        nc.vector.memset(warm[:], 1.0)
        wps = ppool.tile([32, 256], _F32, tag="wps")
        for j in range(32):
            nc.tensor.matmul(
                out=wps[:], lhsT=warm[:, :, 0:32], rhs=warm[:],
                start=(j == 0), stop=(j == 31),
                perf_mode=mybir.MatmulPerfMode.DoubleRow,
            )

        accps = [acc.tile([32, 512], _F32, tag=f"accp{j}", name=f"accp{j}")
                 for j in range(NBLK // 2)]
        accs = [accps[b // 2][:, (b % 2) * 256 : (b % 2 + 1) * 256]
                for b in range(NBLK)]
        STs = [[spool.tile([128, 128], _F32, tag=f"st{m}{h}", name=f"st{m}{h}")
                for h in range(2)] for m in range(2)]

        def emit_head(m: int) -> None:
            # gather ST[m][h][feat, seg] from the 4 finished PSUM accumulators
            for bb in range(4):
                b = 4 * m + bb
                sb = spool.tile([32, 256], _F32, tag="sb")
                nc.scalar.copy(sb[:], accs[b])
                for h in range(2):
                    tp = ppool.tile([128, 32], _F32, tag="tp")
                    nc.tensor.transpose(
                        out=tp[:], in_=sb[:, h * 128 : (h + 1) * 128],
                        identity=ident_t[:],
                    )
                    nc.scalar.copy(STs[m][h][:, bb * 32 : (bb + 1) * 32], tp[:])
            # h' = S @ W1 + counts * b1   (= n_g * (pooled @ W1 + b1))
            h_ps = ppool.tile([128, 128], _F32, tag="h")
            nc.tensor.matmul(
                out=h_ps[:], lhsT=STs[m][0][:], rhs=w1_t[:, 0, :],
                start=True, stop=False,
            )
            nc.tensor.matmul(
                out=h_ps[:], lhsT=STs[m][1][:], rhs=w1_t[:, 1, :],
                start=False, stop=False,
            )
            nc.tensor.matmul(
                out=h_ps[:], lhsT=ct_t[:, m * 128 : (m + 1) * 128], rhs=b1_t[:],
                start=False, stop=True,
            )
            # LayerNorm with eps' = eps * n_g^2 (exact scale compensation)
            musum = spool.tile([128, 1], _F32, tag="musum")
            nc.vector.tensor_reduce(
                out=musum[:], in_=h_ps[:], axis=mybir.AxisListType.X, op=_ALU.add
            )
            mu = spool.tile([128, 1], _F32, tag="mu")
            nc.vector.tensor_scalar(
                out=mu[:], in0=musum[:], scalar1=1.0 / 128, scalar2=None,
                op0=_ALU.mult,
            )
            hc = spool.tile([128, 128], _F32, tag="hc")
            nc.vector.tensor_scalar(
                out=hc[:], in0=h_ps[:], scalar1=mu[:], scalar2=None,
                op0=_ALU.subtract,
            )
            sq = spool.tile([128, 128], _F32, tag="sq")
            ssq = spool.tile([128, 1], _F32, tag="ssq")
            nc.vector.scalar_tensor_tensor(
                out=sq[:], in0=hc[:], scalar=1.0, in1=hc[:],
                op0=_ALU.mult, op1=_ALU.mult, accum_out=ssq[:],
            )
            std = spool.tile([128, 1], _F32, tag="std")
            nc.scalar.activation(
                std[:], ssq[:], _ACT.Sqrt,
                bias=epsg_t[:, m : m + 1], scale=1.0 / 128,
            )
            rstd = spool.tile([128, 1], _F32, tag="rstd")
            nc.vector.reciprocal(rstd[:], std[:])
            y = spool.tile([128, 128], _F32, tag="y")
            nc.vector.scalar_tensor_tensor(
                out=y[:], in0=hc[:], scalar=rstd[:], in1=bc[:, 0:128],
                op0=_ALU.mult, op1=_ALU.mult,
            )
            y2 = spool.tile([128, 128], _F32, tag="y2")
            nc.vector.tensor_tensor(
                out=y2[:], in0=y[:], in1=bc[:, 128:256], op=_ALU.add
            )
            yl = spool.tile([128, 128], _F32, tag="yl")
            nc.vector.scalar_tensor_tensor(
                out=yl[:], in0=y2[:], scalar=NEG_SLOPE, in1=y2[:],
                op0=_ALU.mult, op1=_ALU.max,
            )
            prod = spool.tile([128, 128], _F32, tag="prod")
            oc = spool.tile([128, 1], _F32, tag="oc")
            nc.vector.scalar_tensor_tensor(
                out=prod[:], in0=yl[:], scalar=1.0, in1=bc[:, 256:384],
                op0=_ALU.mult, op1=_ALU.mult, accum_out=oc[:],
            )
            ofin = spool.tile([128, 1], _F32, tag="ofin")
            nc.vector.tensor_scalar(
                out=ofin[:], in0=oc[:], scalar1=bc[:, 384:385], scalar2=None,
                op0=_ALU.add,
            )
            nc.tensor.matmul(
                out=wps[0:1, 0:128], lhsT=ofin[:], rhs=id128_t[:],
                start=True, stop=True,
            )
            orow = spool.tile([1, 128], _F32, tag="orows")
            nc.scalar.copy(orow[:], wps[0:1, 0:128])
            nc.sync.dma_start(out=out_d[m, :], in_=orow[:])

        # ---- main stream ----
        head0_emitted = False
        for c in range(NCH):
            ft = fpool.tile([128, K2, 2, 256], _F8, tag="ft")
            dma_eng = (nc.sync, nc.scalar, nc.gpsimd)[c % 3]
            dma_eng.dma_start(
                out=ft[:],
                in_=feat_d[c].rearrange("p (k i f) -> p k i f", k=K2, i=2),
            )
            oh = opool.tile([128, 2 * K2, W], _F8, tag="oh")
            nc.vector.tensor_tensor(
                out=oh[:],
                in0=iota_t[:],
                in1=segT_t[:, 2 * K2 * c : 2 * K2 * (c + 1)].unsqueeze(2).broadcast_to(
                    [128, 2 * K2, W]),
                op=_ALU.is_equal,
            )
            for k in range(K2):
                g = K2 * c + k
                b = g // G
                nc.tensor.matmul(
                    out=accs[b],
                    lhsT=oh[:, 2 * k : 2 * k + 2, :],
                    rhs=ft[:, k, :, :],
                    start=(g % G == 0),
                    stop=(g % G == G - 1),
                    perf_mode=mybir.MatmulPerfMode.DoubleRow,
                )
            done = K2 * (c + 1)
            if not head0_emitted and done >= 4 * G:
                emit_head(0)
                head0_emitted = True
        emit_head(1)

    _split_excess_waits(nc)
    return nc


def _ef_quantize(feats: np.ndarray) -> np.ndarray:
    """fp8(e4m3) quantization with error feedback along the node axis.

    Carry chains run over consecutive nodes (cut every 128 for
    vectorization); since batch ids are sorted, per-segment sums of the
    quantized stream telescope to the exact sum plus O(1 ulp) carry terms.
    """
    n, d = feats.shape
    nchain = (n + 127) // 128
    pad = nchain * 128 - n
    x = np.concatenate([feats, np.zeros((pad, d), np.float32)], axis=0) if pad else feats
    x = x.reshape(nchain, 128, d)
    q = np.empty((nchain, 128, d), _FP8NP)
    c = np.zeros((nchain, d), np.float32)
    for p in range(128):
        y = x[:, p, :] + c
        qp = y.astype(_FP8NP)
        q[:, p, :] = qp
        c = y - qp.astype(np.float32)
    return q.reshape(nchain * 128, d)[:n]


def _prep_inputs(features, batch):
    feats = np.ascontiguousarray(np.asarray(features), dtype=np.float32)
    seg = np.asarray(batch).astype(np.int64)
    n = seg.shape[0]
    counts = np.bincount(seg, minlength=NUM_GRAPHS)
    bnd = np.zeros(NUM_GRAPHS + 1, np.int64)
    bnd[1:] = np.cumsum(counts)
    blk_lo = bnd[0:NUM_GRAPHS:W]  # [64]
    blk_hi = bnd[W : NUM_GRAPHS + 1 : W]
    blk_n = blk_hi - blk_lo
    G = int(max(32, np.max((blk_n + 255) // 256)))
    if (8 * G) % K2 != 0:
        G += (K2 - (8 * G) % K2 + 7) // 8  # bump G so 8G is a K2 multiple

    q = _ef_quantize(feats)  # [n, 256] fp8

    # node (core, g, ii, p) -> global node id, valid mask
    core = np.arange(N_CORES)[:, None, None, None]
    g = np.arange(NBLK * G)[None, :, None, None]
    ii = np.arange(2)[None, None, :, None]
    p = np.arange(128)[None, None, None, :]
    b = g // G
    gb = NBLK * core + b  # global block id [8, 8G, 1, 1]
    off = (g - b * G) * 256 + ii * 128 + p
    node = blk_lo[gb] + off
    valid = node < blk_hi[gb]
    node_c = np.minimum(node, n - 1)

    feat_dev = q[node_c]  # [8, 8G, 2, 128, 256] fp8
    feat_dev[~valid] = 0
    # device layout [core, chunk, p, k, ii, f]
    nch = 8 * G // K2
    feat_dev = feat_dev.reshape(N_CORES, nch, K2, 2, 128, 256).transpose(0, 1, 4, 2, 3, 5)
    feat_dev = np.ascontiguousarray(feat_dev).reshape(N_CORES, nch, 128, K2 * 2 * 256)

    segv = np.where(valid, seg[node_c] - W * gb, -1).astype(np.float32)  # [8, 8G, 2, 128]
    # segT[core, p, 2g+ii]
    segT = np.ascontiguousarray(segv.transpose(0, 3, 1, 2)).reshape(N_CORES, 128, -1)
    segT = segT.astype(ml_dtypes.bfloat16)
    return feat_dev, segT, counts, G


def kernel(features, batch, W1, b1, gamma, beta, W2, b2):
    feat_dev, segT, counts, G = _prep_inputs(features, batch)

    iota = np.tile(
        np.arange(W, dtype=np.float32)[None, None, :], (128, 2 * K2, 1)
    ).astype(ml_dtypes.bfloat16)
    ident = np.eye(32, dtype=np.float32)
    id128 = np.eye(128, dtype=np.float32)
    w1 = np.ascontiguousarray(
        np.asarray(W1, np.float32).reshape(2, 128, 128).transpose(1, 0, 2)
    )
    b1row = np.asarray(b1, np.float32).reshape(1, 128)
    pvec = np.concatenate(
        [
            np.asarray(gamma, np.float32).ravel(),
            np.asarray(beta, np.float32).ravel(),
            np.asarray(W2, np.float32).ravel(),
            np.asarray(b2, np.float32).ravel(),
        ]
    )[None, :]

    ct = counts.astype(np.float32).reshape(N_CORES, 1, SEGS_PER_CORE)
    epsg = (
        LN_EPS * np.maximum(counts.astype(np.float32), 1.0) ** 2
    ).reshape(N_CORES, 2, 128).transpose(0, 2, 1)
    epsg = np.ascontiguousarray(epsg)

    if G not in _PROGRAM_CACHE:
        _PROGRAM_CACHE[G] = _build_program(G)
    nc = _PROGRAM_CACHE[G]
    in_maps = [
        {
            "feat": feat_dev[i],
            "segT": segT[i],
            "iota": iota,
            "ident": ident,
            "id128": id128,
            "w1": w1,
            "b1row": b1row,
            "countsT": ct[i],
            "epsg": epsg[i],
            "pvec": pvec,
        }
        for i in range(N_CORES)
    ]
    res = run_bass_kernel_spmd(
        nc, in_maps, list(range(N_CORES)), trace=PROFILE, tmpdir=PROFILE_DIR
    )
    global LAST_RESULT
    LAST_RESULT = res
    out = np.concatenate(
        [res.results[i]["out"].reshape(SEGS_PER_CORE) for i in range(N_CORES)]
    )
    return out.reshape(NUM_GRAPHS, 1).astype(np.float32)


# revision 12
# speedup vs baseline: 1.0612x; 1.0612x over previous
"""Trainium2 Bass kernel for nn_Classifier (segment mean-pool + tiny MLP head).

Pipeline (matches the jax reference):
  pooled[g] = mean of features over nodes with batch id g   (2048 graphs)
  out = LeakyReLU(LayerNorm(pooled @ W1 + b1)) @ W2 + b2    -> [2048, 1]

Design (v2, tuned from hardware microbenchmarks):
  * Data-parallel over 8 cores at 32-segment block granularity: core i owns
    graphs [256i, 256i+256) = 8 blocks of 32 segments, and exactly the nodes
    belonging to them (batch ids are sorted).
  * Features are quantized host-side to fp8 (e4m3) with error feedback along
    the node order: q_i = Q(x_i + c_{i-1}), c_i = x_i + c_{i-1} - q_i. Segment
    sums of q then telescope, so the quantization error per (segment, feature)
    sum is bounded by two carry terms (~0.04 std) instead of sqrt(n)*fp8
    noise. Halves HBM traffic vs bf16; DMA is the roofline.
  * Segment sums via one-hot matmul on the tensor engine in fp8 DoubleRow
    mode: each matmul contracts 256 nodes (2 k-tiles x 128 partitions)
    against a [128, 2, 32] one-hot, accumulating [32 segs, 256 feats] in
    PSUM per block. ~109 ns/matmul measured (issue-rate bound).
  * One-hots are built on the vector engine in batches of 8 groups with a
    single tensor_tensor is_equal (iota vs broadcast seg ids, bf16 in ->
    fp8 out). DVE measured ~1.2 ns/elem; 32-wide windows keep this off the
    critical path. GpSimd/Act are avoided (measured 2169/800 ns per tile).
  * No division by counts: the head computes h' = n_g * h and LayerNorm
    with a per-graph eps' = eps * max(n_g,1)^2, which is exactly equivalent
    (LN is scale-invariant up to the eps term). b1 enters as counts x b1
    via a K=1 matmul, so the head matches the reference for any b1.
  * The MLP head for segments [0,128) runs mid-stream once blocks 0-3 are
    done; the head for [128,256) runs at the end.
"""

from contextlib import ExitStack

import numpy as np
import ml_dtypes

import concourse.bass as bass
import concourse.mybir as mybir
import concourse.tile as tile
from concourse.bass_utils import run_bass_kernel_spmd

# ---------------------------------------------------------------------------
# Workaround: this walrus build rejects instructions carrying more than one
# semaphore wait ("Too many sync wait commands"), but Tile's semaphore
# assignment freely attaches several. After the TileContext has lowered the
# program, split any excess waits onto same-engine nops inserted right before
# the instruction (semantics are identical: all waits are monotonic and must
# hold before the instruction issues).
_MAX_WAITS = 1


def _split_excess_waits(nc: "bass.Bass", max_waits: int = _MAX_WAITS) -> None:
    ctr = 0
    for f in nc.m.functions:
        for b in f.blocks:
            out = []
            for inst in b.instructions:
                si = inst.sync_info
                waits = list(si.on_wait) if (si is not None and si.on_wait) else []
                if len(waits) > max_waits:
                    keep = waits[-max_waits:]
                    extra = waits[:-max_waits]
                    # On the PE queue the carrier must be a DRAIN: silicon
                    # promotes waitless LDWEIGHTS past in-flight work, so a
                    # plain nop's wait can be bypassed (walrus attaches a
                    # matmul's waits to its LDWEIGHTS — stripping them onto a
                    # nop re-opens that race). A drain fully serializes.
                    is_pe = inst.engine == mybir.EngineType.PE
                    for i in range(0, len(extra), max_waits):
                        ctr += 1
                        if is_pe:
                            nop = mybir.InstDrain(
                                name=f"waitsplit_drain_{ctr}", ins=[], outs=[],
                                engine=inst.engine,
                            )
                        else:
                            nop = mybir.InstNoOp(
                                name=f"waitsplit_nop_{ctr}", ins=[], outs=[],
                                engine=inst.engine,
                            )
                        nop.sync_info = mybir.SyncInfo(
                            on_wait=extra[i : i + max_waits], on_update=[]
                        )
                        nc.register_instruction(nop)
                        out.append(nop)
                    inst.sync_info = mybir.SyncInfo(
                        on_wait=keep, on_update=list(si.on_update or [])
                    )
                out.append(inst)
            b.instructions = out
# ---------------------------------------------------------------------------

N_CORES = 8
NUM_GRAPHS = 2048
SEGS_PER_CORE = NUM_GRAPHS // N_CORES  # 256
W = 32  # segment-block width (one PSUM accumulator per block)
NBLK = SEGS_PER_CORE // W  # 8 blocks per core
D = 256
K2 = 16  # 256-node groups per DMA chunk (chunk = 4096 nodes, 1 MB fp8)
LN_EPS = 1e-5
NEG_SLOPE = 0.01

_F32 = mybir.dt.float32
_BF16 = mybir.dt.bfloat16
_F8 = mybir.dt.float8e4
_ALU = mybir.AluOpType
_ACT = mybir.ActivationFunctionType
_FP8NP = ml_dtypes.float8_e4m3fn

# Test/debug hooks: set PROFILE=True before calling kernel() to request an
# NTFF trace; the BassKernelResults lands in LAST_RESULT.
PROFILE = False
PROFILE_DIR = None
LAST_RESULT = None

_PROGRAM_CACHE = {}


def _build_program(G: int) -> bass.Bass:
    """G = groups (of 256 nodes) per 32-segment block."""
    assert (8 * G) % K2 == 0
    NCH = 8 * G // K2
    nc = bass.Bass("TRN2", debug=False)
    feat_d = nc.dram_tensor("feat", [NCH, 128, K2 * 2 * 256], _F8, kind="ExternalInput").ap()
    segT_d = nc.dram_tensor("segT", [128, 16 * G], _BF16, kind="ExternalInput").ap()
    iota_d = nc.dram_tensor("iota", [128, 2 * K2, W], _BF16, kind="ExternalInput").ap()
    blob_d = nc.dram_tensor("blob", [128, 1187], _F32, kind="ExternalInput").ap()
    out_d = nc.dram_tensor("out", [2, 128], _F32, kind="ExternalOutput").ap()

    with tile.TileContext(nc) as tc, ExitStack() as ctx:
        cpool = ctx.enter_context(tc.tile_pool(name="consts", bufs=1))
        fpool = ctx.enter_context(tc.tile_pool(name="feat", bufs=6))
        opool = ctx.enter_context(tc.tile_pool(name="oh", bufs=8))
        acc = ctx.enter_context(tc.tile_pool(name="acc", bufs=1, space="PSUM"))
        ppool = ctx.enter_context(tc.tile_pool(name="pw", bufs=1, space="PSUM"))
        spool = ctx.enter_context(tc.tile_pool(name="small", bufs=2))

        segT_t = cpool.tile([128, 16 * G], _BF16, tag="segT")
        nc.gpsimd.dma_start(out=segT_t[:], in_=segT_d[:])
        iota_t = cpool.tile([128, 2 * K2, W], _BF16, tag="iota")
        nc.gpsimd.dma_start(out=iota_t[:], in_=iota_d[:])
        blob_t = cpool.tile([128, 1187], _F32, tag="blob")
        nc.gpsimd.dma_start(out=blob_t[:], in_=blob_d[:])
        w1_t = blob_t[:, 0:256].rearrange("p (h j) -> p h j", h=2)
        id128_t = blob_t[:, 256:384]
        ident_t = blob_t[0:32, 384:416]
        epsg_t = blob_t[:, 416:418]
        b1_t = blob_t[0:1, 418:546]
        ct_t = blob_t[0:1, 546:802]
        pv_t = blob_t[0:1, 802:1187]
        ones_row = cpool.tile([1, 128], _F32, tag="ones")
        nc.vector.memset(ones_row[:], 1.0)

        # broadcast [gamma | beta | W2 | b2] to all 128 partitions
        bc_ps = ppool.tile([128, 385], _F32, tag="bc")
        nc.tensor.matmul(
            out=bc_ps[:], lhsT=ones_row[:], rhs=pv_t, start=True, stop=True
        )
        bc = cpool.tile([128, 385], _F32, tag="bcs")
        nc.scalar.copy(bc[:], bc_ps[:])

        warm = cpool.tile([128, 2, 256], _F8, tag="warm")
        nc.vector.memset(warm[:], 1.0)
        wps = ppool.tile([32, 256], _F32, tag="wps")
        for j in range(8):
            nc.tensor.matmul(
                out=wps[:], lhsT=warm[:, :, 0:32], rhs=warm[:],
                start=(j == 0), stop=(j == 7),
                perf_mode=mybir.MatmulPerfMode.DoubleRow,
            )

        accps = [acc.tile([32, 512], _F32, tag=f"accp{j}", name=f"accp{j}")
                 for j in range(NBLK // 2)]
        accs = [accps[b // 2][:, (b % 2) * 256 : (b % 2 + 1) * 256]
                for b in range(NBLK)]
        STs = [[spool.tile([128, 128], _F32, tag=f"st{m}{h}", name=f"st{m}{h}")
                for h in range(2)] for m in range(2)]

        sbs = [None] * NBLK

        def head_stage_a(m: int) -> None:
            # PSUM -> SBUF copies of the 4 finished accumulators (Act)
            for bb in range(4):
                b = 4 * m + bb
                sb = spool.tile([32, 256], _F32, tag=f"sb{b}", name=f"sb{b}")
                nc.scalar.copy(sb[:], accs[b])
                sbs[b] = sb

        def head_stage_b(m: int) -> None:
            # transpose to ST[m][h][feat, seg] (PE + Act, inputs already ready)
            for bb in range(4):
                b = 4 * m + bb
                for h in range(2):
                    tp = ppool.tile([128, 32], _F32, tag="tp")
                    nc.tensor.transpose(
                        out=tp[:], in_=sbs[b][:, h * 128 : (h + 1) * 128],
                        identity=ident_t,
                    )
                    nc.scalar.copy(STs[m][h][:, bb * 32 : (bb + 1) * 32], tp[:])

        def head_stage_c(m: int) -> None:
            # h' = S @ W1 + counts * b1   (= n_g * (pooled @ W1 + b1))
            h_ps = ppool.tile([128, 128], _F32, tag="h")
            nc.tensor.matmul(
                out=h_ps[:], lhsT=STs[m][0][:], rhs=w1_t[:, 0, :],
                start=True, stop=False,
            )
            nc.tensor.matmul(
                out=h_ps[:], lhsT=STs[m][1][:], rhs=w1_t[:, 1, :],
                start=False, stop=False,
            )
            nc.tensor.matmul(
                out=h_ps[:], lhsT=ct_t[0:1, m * 128 : (m + 1) * 128], rhs=b1_t,
                start=False, stop=True,
            )
            # LayerNorm with eps' = eps * n_g^2 (exact scale compensation)
            musum = spool.tile([128, 1], _F32, tag="musum")
            nc.vector.tensor_reduce(
                out=musum[:], in_=h_ps[:], axis=mybir.AxisListType.X, op=_ALU.add
            )
            mu = spool.tile([128, 1], _F32, tag="mu")
            nc.vector.tensor_scalar(
                out=mu[:], in0=musum[:], scalar1=1.0 / 128, scalar2=None,
                op0=_ALU.mult,
            )
            hc = spool.tile([128, 128], _F32, tag="hc")
            nc.vector.tensor_scalar(
                out=hc[:], in0=h_ps[:], scalar1=mu[:], scalar2=None,
                op0=_ALU.subtract,
            )
            sq = spool.tile([128, 128], _F32, tag="sq")
            ssq = spool.tile([128, 1], _F32, tag="ssq")
            nc.vector.scalar_tensor_tensor(
                out=sq[:], in0=hc[:], scalar=1.0, in1=hc[:],
                op0=_ALU.mult, op1=_ALU.mult, accum_out=ssq[:],
            )
            std = spool.tile([128, 1], _F32, tag="std")
            nc.scalar.activation(
                std[:], ssq[:], _ACT.Sqrt,
                bias=epsg_t[:, m : m + 1], scale=1.0 / 128,
            )
            rstd = spool.tile([128, 1], _F32, tag="rstd")
            nc.vector.reciprocal(rstd[:], std[:])
            y = spool.tile([128, 128], _F32, tag="y")
            nc.vector.scalar_tensor_tensor(
                out=y[:], in0=hc[:], scalar=rstd[:], in1=bc[:, 0:128],
                op0=_ALU.mult, op1=_ALU.mult,
            )
            y2 = spool.tile([128, 128], _F32, tag="y2")
            nc.vector.tensor_tensor(
                out=y2[:], in0=y[:], in1=bc[:, 128:256], op=_ALU.add
            )
            yl = spool.tile([128, 128], _F32, tag="yl")
            nc.vector.scalar_tensor_tensor(
                out=yl[:], in0=y2[:], scalar=NEG_SLOPE, in1=y2[:],
                op0=_ALU.mult, op1=_ALU.max,
            )
            prod = spool.tile([128, 128], _F32, tag="prod")
            oc = spool.tile([128, 1], _F32, tag="oc")
            nc.vector.scalar_tensor_tensor(
                out=prod[:], in0=yl[:], scalar=1.0, in1=bc[:, 256:384],
                op0=_ALU.mult, op1=_ALU.mult, accum_out=oc[:],
            )
            ofin = spool.tile([128, 1], _F32, tag="ofin")
            nc.vector.tensor_scalar(
                out=ofin[:], in0=oc[:], scalar1=bc[:, 384:385], scalar2=None,
                op0=_ALU.add,
            )
            nc.tensor.matmul(
                out=wps[0:1, 0:128], lhsT=ofin[:], rhs=id128_t,
                start=True, stop=True,
            )
            orow = spool.tile([1, 128], _F32, tag="orows")
            nc.scalar.copy(orow[:], wps[0:1, 0:128])
            nc.sync.dma_start(out=out_d[m, :], in_=orow[:])

        # ---- main stream ----
        schedule = {}
        for m in (0, 1):
            cA = -((-(4 * (m + 1)) * G) // K2) - 1  # chunk closing block 4m+3
            schedule.setdefault(cA, []).append(lambda m=m: head_stage_a(m))
            schedule.setdefault(cA + 2, []).append(lambda m=m: head_stage_b(m))
            schedule.setdefault(cA + 4, []).append(lambda m=m: head_stage_c(m))
        for c in range(NCH):
            ft = fpool.tile([128, K2, 2, 256], _F8, tag="ft")
            dma_eng = (nc.sync, nc.scalar)[c % 2]
            dma_eng.dma_start(
                out=ft[:],
                in_=feat_d[c].rearrange("p (k i f) -> p k i f", k=K2, i=2),
            )
            oh = opool.tile([128, 2 * K2, W], _F8, tag="oh")
            nc.vector.tensor_tensor(
                out=oh[:],
                in0=iota_t[:],
                in1=segT_t[:, 2 * K2 * c : 2 * K2 * (c + 1)].unsqueeze(2).broadcast_to(
                    [128, 2 * K2, W]),
                op=_ALU.is_equal,
            )
            for k in range(K2):
                g = K2 * c + k
                b = g // G
                nc.tensor.matmul(
                    out=accs[b],
                    lhsT=oh[:, 2 * k : 2 * k + 2, :],
                    rhs=ft[:, k, :, :],
                    start=(g % G == 0),
                    stop=(g % G == G - 1),
                    perf_mode=mybir.MatmulPerfMode.DoubleRow,
                )
            for fn in schedule.pop(c, []):
                fn()
        for c in sorted(schedule):
            for fn in schedule[c]:
                fn()

    _split_excess_waits(nc)
    return nc


def _ef_quantize(feats: np.ndarray) -> np.ndarray:
    """fp8(e4m3) quantization with error feedback along the node axis.

    Carry chains run over consecutive nodes (cut every 128 for
    vectorization); since batch ids are sorted, per-segment sums of the
    quantized stream telescope to the exact sum plus O(1 ulp) carry terms.
    """
    n, d = feats.shape
    nchain = (n + 127) // 128
    pad = nchain * 128 - n
    x = np.concatenate([feats, np.zeros((pad, d), np.float32)], axis=0) if pad else feats
    x = x.reshape(nchain, 128, d)
    q = np.empty((nchain, 128, d), _FP8NP)
    c = np.zeros((nchain, d), np.float32)
    for p in range(128):
        y = x[:, p, :] + c
        qp = y.astype(_FP8NP)
        q[:, p, :] = qp
        c = y - qp.astype(np.float32)
    return q.reshape(nchain * 128, d)[:n]


def _prep_inputs(features, batch):
    feats = np.ascontiguousarray(np.asarray(features), dtype=np.float32)
    seg = np.asarray(batch).astype(np.int64)
    n = seg.shape[0]
    counts = np.bincount(seg, minlength=NUM_GRAPHS)
    bnd = np.zeros(NUM_GRAPHS + 1, np.int64)
    bnd[1:] = np.cumsum(counts)
    blk_lo = bnd[0:NUM_GRAPHS:W]  # [64]
    blk_hi = bnd[W : NUM_GRAPHS + 1 : W]
    blk_n = blk_hi - blk_lo
    G = int(max(32, np.max((blk_n + 255) // 256)))
    if (8 * G) % K2 != 0:
        G += (K2 - (8 * G) % K2 + 7) // 8  # bump G so 8G is a K2 multiple

    q = _ef_quantize(feats)  # [n, 256] fp8

    # node (core, g, ii, p) -> global node id, valid mask
    core = np.arange(N_CORES)[:, None, None, None]
    g = np.arange(NBLK * G)[None, :, None, None]
    ii = np.arange(2)[None, None, :, None]
    p = np.arange(128)[None, None, None, :]
    b = g // G
    gb = NBLK * core + b  # global block id [8, 8G, 1, 1]
    off = (g - b * G) * 256 + ii * 128 + p
    node = blk_lo[gb] + off
    valid = node < blk_hi[gb]
    node_c = np.minimum(node, n - 1)

    feat_dev = q[node_c]  # [8, 8G, 2, 128, 256] fp8
    feat_dev[~valid] = 0
    # device layout [core, chunk, p, k, ii, f]
    nch = 8 * G // K2
    feat_dev = feat_dev.reshape(N_CORES, nch, K2, 2, 128, 256).transpose(0, 1, 4, 2, 3, 5)
    feat_dev = np.ascontiguousarray(feat_dev).reshape(N_CORES, nch, 128, K2 * 2 * 256)

    segv = np.where(valid, seg[node_c] - W * gb, -1).astype(np.float32)  # [8, 8G, 2, 128]
    # segT[core, p, 2g+ii]
    segT = np.ascontiguousarray(segv.transpose(0, 3, 1, 2)).reshape(N_CORES, 128, -1)
    segT = segT.astype(ml_dtypes.bfloat16)
    return feat_dev, segT, counts, G


def kernel(features, batch, W1, b1, gamma, beta, W2, b2):
    feat_dev, segT, counts, G = _prep_inputs(features, batch)

    iota = np.tile(
        np.arange(W, dtype=np.float32)[None, None, :], (128, 2 * K2, 1)
    ).astype(ml_dtypes.bfloat16)

    blob = np.zeros((N_CORES, 128, 1187), np.float32)
    w1 = np.asarray(W1, np.float32).reshape(2, 128, 128).transpose(1, 0, 2)
    blob[:, :, 0:256] = w1.reshape(128, 256)[None]
    blob[:, :, 256:384] = np.eye(128, dtype=np.float32)[None]
    blob[:, 0:32, 384:416] = np.eye(32, dtype=np.float32)[None]
    epsg = (
        LN_EPS * np.maximum(counts.astype(np.float32), 1.0) ** 2
    ).reshape(N_CORES, 2, 128).transpose(0, 2, 1)
    blob[:, :, 416:418] = epsg
    blob[:, 0, 418:546] = np.asarray(b1, np.float32).ravel()[None]
    blob[:, 0, 546:802] = counts.astype(np.float32).reshape(N_CORES, 256)
    blob[:, 0, 802:1187] = np.concatenate(
        [
            np.asarray(gamma, np.float32).ravel(),
            np.asarray(beta, np.float32).ravel(),
            np.asarray(W2, np.float32).ravel(),
            np.asarray(b2, np.float32).ravel(),
        ]
    )[None]

    if G not in _PROGRAM_CACHE:
        _PROGRAM_CACHE[G] = _build_program(G)
    nc = _PROGRAM_CACHE[G]
    in_maps = [
        {
            "feat": feat_dev[i],
            "segT": segT[i],
            "iota": iota,
            "blob": blob[i],
        }
        for i in range(N_CORES)
    ]
    res = run_bass_kernel_spmd(
        nc, in_maps, list(range(N_CORES)), trace=PROFILE, tmpdir=PROFILE_DIR
    )
    global LAST_RESULT
    LAST_RESULT = res
    out = np.concatenate(
        [res.results[i]["out"].reshape(SEGS_PER_CORE) for i in range(N_CORES)]
    )
    return out.reshape(NUM_GRAPHS, 1).astype(np.float32)


# revision 13
# speedup vs baseline: 1.1390x; 1.0733x over previous
"""Trainium2 Bass kernel for nn_Classifier (segment mean-pool + tiny MLP head).

Pipeline (matches the jax reference):
  pooled[g] = mean of features over nodes with batch id g   (2048 graphs)
  out = LeakyReLU(LayerNorm(pooled @ W1 + b1)) @ W2 + b2    -> [2048, 1]

Design (v2, tuned from hardware microbenchmarks):
  * Data-parallel over 8 cores at 32-segment block granularity: core i owns
    graphs [256i, 256i+256) = 8 blocks of 32 segments, and exactly the nodes
    belonging to them (batch ids are sorted).
  * Features are quantized host-side to fp8 (e4m3) with error feedback along
    the node order: q_i = Q(x_i + c_{i-1}), c_i = x_i + c_{i-1} - q_i. Segment
    sums of q then telescope, so the quantization error per (segment, feature)
    sum is bounded by two carry terms (~0.04 std) instead of sqrt(n)*fp8
    noise. Halves HBM traffic vs bf16; DMA is the roofline.
  * Segment sums via one-hot matmul on the tensor engine in fp8 DoubleRow
    mode: each matmul contracts 256 nodes (2 k-tiles x 128 partitions)
    against a [128, 2, 32] one-hot, accumulating [32 segs, 256 feats] in
    PSUM per block. ~109 ns/matmul measured (issue-rate bound).
  * One-hots are built on the vector engine in batches of 8 groups with a
    single tensor_tensor is_equal (iota vs broadcast seg ids, bf16 in ->
    fp8 out). DVE measured ~1.2 ns/elem; 32-wide windows keep this off the
    critical path. GpSimd/Act are avoided (measured 2169/800 ns per tile).
  * No division by counts: the head computes h' = n_g * h and LayerNorm
    with a per-graph eps' = eps * max(n_g,1)^2, which is exactly equivalent
    (LN is scale-invariant up to the eps term). b1 enters as counts x b1
    via a K=1 matmul, so the head matches the reference for any b1.
  * The MLP head for segments [0,128) runs mid-stream once blocks 0-3 are
    done; the head for [128,256) runs at the end.
"""

from contextlib import ExitStack

import numpy as np
import ml_dtypes

import concourse.bass as bass
import concourse.mybir as mybir
import concourse.tile as tile
from concourse.bass_utils import run_bass_kernel_spmd

# ---------------------------------------------------------------------------
# Workaround: this walrus build rejects instructions carrying more than one
# semaphore wait ("Too many sync wait commands"), but Tile's semaphore
# assignment freely attaches several. After the TileContext has lowered the
# program, split any excess waits onto same-engine nops inserted right before
# the instruction (semantics are identical: all waits are monotonic and must
# hold before the instruction issues).
_MAX_WAITS = 1


def _split_excess_waits(nc: "bass.Bass", max_waits: int = _MAX_WAITS) -> None:
    ctr = 0
    for f in nc.m.functions:
        for b in f.blocks:
            out = []
            for inst in b.instructions:
                si = inst.sync_info
                waits = list(si.on_wait) if (si is not None and si.on_wait) else []
                if len(waits) > max_waits:
                    keep = waits[-max_waits:]
                    extra = waits[:-max_waits]
                    # On the PE queue the carrier must be a DRAIN: silicon
                    # promotes waitless LDWEIGHTS past in-flight work, so a
                    # plain nop's wait can be bypassed (walrus attaches a
                    # matmul's waits to its LDWEIGHTS — stripping them onto a
                    # nop re-opens that race). A drain fully serializes.
                    is_pe = inst.engine == mybir.EngineType.PE
                    for i in range(0, len(extra), max_waits):
                        ctr += 1
                        if is_pe:
                            nop = mybir.InstDrain(
                                name=f"waitsplit_drain_{ctr}", ins=[], outs=[],
                                engine=inst.engine,
                            )
                        else:
                            nop = mybir.InstNoOp(
                                name=f"waitsplit_nop_{ctr}", ins=[], outs=[],
                                engine=inst.engine,
                            )
                        nop.sync_info = mybir.SyncInfo(
                            on_wait=extra[i : i + max_waits], on_update=[]
                        )
                        nc.register_instruction(nop)
                        out.append(nop)
                    inst.sync_info = mybir.SyncInfo(
                        on_wait=keep, on_update=list(si.on_update or [])
                    )
                out.append(inst)
            b.instructions = out
# ---------------------------------------------------------------------------

N_CORES = 8
NUM_GRAPHS = 2048
SEGS_PER_CORE = NUM_GRAPHS // N_CORES  # 256
W = 32  # segment-block width (one PSUM accumulator per block)
NBLK = SEGS_PER_CORE // W  # 8 blocks per core
D = 256
K2 = 16  # 256-node groups per DMA chunk (chunk = 4096 nodes, 1 MB fp8)
LN_EPS = 1e-5
NEG_SLOPE = 0.01

_F32 = mybir.dt.float32
_BF16 = mybir.dt.bfloat16
_F8 = mybir.dt.float8e4
_ALU = mybir.AluOpType
_ACT = mybir.ActivationFunctionType
_FP8NP = ml_dtypes.float8_e4m3fn

# Test/debug hooks: set PROFILE=True before calling kernel() to request an
# NTFF trace; the BassKernelResults lands in LAST_RESULT.
PROFILE = False
PROFILE_DIR = None
LAST_RESULT = None

_PROGRAM_CACHE = {}


def _build_program(G: int) -> bass.Bass:
    """G = groups (of 256 nodes) per 32-segment block."""
    assert (8 * G) % K2 == 0
    NCH = 8 * G // K2
    nc = bass.Bass("TRN2", debug=False)
    feat_d = nc.dram_tensor("feat", [NCH, 128, K2 * 2 * 256], _F8, kind="ExternalInput").ap()
    segT_d = nc.dram_tensor("segT", [128, 16 * G], _BF16, kind="ExternalInput").ap()
    iota_d = nc.dram_tensor("iota", [128, 2 * K2, W], _BF16, kind="ExternalInput").ap()
    blob_d = nc.dram_tensor("blob", [128, 1187], _F32, kind="ExternalInput").ap()
    out_d = nc.dram_tensor("out", [2, 128], _F32, kind="ExternalOutput").ap()

    with tile.TileContext(nc) as tc, ExitStack() as ctx:
        cpool = ctx.enter_context(tc.tile_pool(name="consts", bufs=1))
        fpool = ctx.enter_context(tc.tile_pool(name="feat", bufs=8))
        opool = ctx.enter_context(tc.tile_pool(name="oh", bufs=8))
        acc = ctx.enter_context(tc.tile_pool(name="acc", bufs=1, space="PSUM"))
        ppool = ctx.enter_context(tc.tile_pool(name="pw", bufs=1, space="PSUM"))
        spool = ctx.enter_context(tc.tile_pool(name="small", bufs=2))

        segT_t = cpool.tile([128, 16 * G], _BF16, tag="segT")
        nc.sync.dma_start(out=segT_t[:], in_=segT_d[:])
        iota_t = cpool.tile([128, 2 * K2, W], _BF16, tag="iota")
        nc.scalar.dma_start(out=iota_t[:], in_=iota_d[:])
        blob_t = cpool.tile([128, 1187], _F32, tag="blob")
        nc.gpsimd.dma_start(out=blob_t[:], in_=blob_d[:])
        w1_t = blob_t[:, 0:256].rearrange("p (h j) -> p h j", h=2)
        id128_t = blob_t[:, 256:384]
        ident_t = blob_t[0:32, 384:416]
        epsg_t = blob_t[:, 416:418]
        b1_t = blob_t[0:1, 418:546]
        ct_t = blob_t[0:1, 546:802]
        pv_t = blob_t[0:1, 802:1187]
        ones_row = cpool.tile([1, 128], _F32, tag="ones")
        nc.vector.memset(ones_row[:], 1.0)

        # broadcast [gamma | beta | W2 | b2] to all 128 partitions
        bc_ps = ppool.tile([128, 385], _F32, tag="bc")
        nc.tensor.matmul(
            out=bc_ps[:], lhsT=ones_row[:], rhs=pv_t, start=True, stop=True
        )
        bc = cpool.tile([128, 385], _F32, tag="bcs")
        nc.scalar.copy(bc[:], bc_ps[:])

        warm = cpool.tile([128, 2, 256], _F8, tag="warm")
        nc.vector.memset(warm[:], 1.0)
        wps = ppool.tile([32, 256], _F32, tag="wps")
        for j in range(8):
            nc.tensor.matmul(
                out=wps[:], lhsT=warm[:, :, 0:32], rhs=warm[:],
                start=(j == 0), stop=(j == 7),
                perf_mode=mybir.MatmulPerfMode.DoubleRow,
            )

        accps = [acc.tile([32, 512], _F32, tag=f"accp{j}", name=f"accp{j}")
                 for j in range(NBLK // 2)]
        accs = [accps[b // 2][:, (b % 2) * 256 : (b % 2 + 1) * 256]
                for b in range(NBLK)]
        STs = [[spool.tile([128, 128], _F32, tag=f"st{m}{h}", name=f"st{m}{h}")
                for h in range(2)] for m in range(2)]

        sbs = [None] * NBLK

        def head_stage_a(m: int) -> None:
            # PSUM -> SBUF copies of the 4 finished accumulators (Act)
            for bb in range(4):
                b = 4 * m + bb
                sb = spool.tile([32, 256], _F32, tag=f"sb{b}", name=f"sb{b}")
                nc.scalar.copy(sb[:], accs[b])
                sbs[b] = sb

        def head_stage_b(m: int) -> None:
            # transpose to ST[m][h][feat, seg] (PE + Act, inputs already ready)
            for bb in range(4):
                b = 4 * m + bb
                for h in range(2):
                    tp = ppool.tile([128, 32], _F32, tag="tp")
                    nc.tensor.transpose(
                        out=tp[:], in_=sbs[b][:, h * 128 : (h + 1) * 128],
                        identity=ident_t,
                    )
                    nc.scalar.copy(STs[m][h][:, bb * 32 : (bb + 1) * 32], tp[:])

        def head_stage_c(m: int) -> None:
            # h' = S @ W1 + counts * b1   (= n_g * (pooled @ W1 + b1))
            h_ps = ppool.tile([128, 128], _F32, tag="h")
            nc.tensor.matmul(
                out=h_ps[:], lhsT=STs[m][0][:], rhs=w1_t[:, 0, :],
                start=True, stop=False,
            )
            nc.tensor.matmul(
                out=h_ps[:], lhsT=STs[m][1][:], rhs=w1_t[:, 1, :],
                start=False, stop=False,
            )
            nc.tensor.matmul(
                out=h_ps[:], lhsT=ct_t[0:1, m * 128 : (m + 1) * 128], rhs=b1_t,
                start=False, stop=True,
            )
            # LayerNorm with eps' = eps * n_g^2 (exact scale compensation)
            musum = spool.tile([128, 1], _F32, tag="musum")
            nc.vector.tensor_reduce(
                out=musum[:], in_=h_ps[:], axis=mybir.AxisListType.X, op=_ALU.add
            )
            mu = spool.tile([128, 1], _F32, tag="mu")
            nc.vector.tensor_scalar(
                out=mu[:], in0=musum[:], scalar1=1.0 / 128, scalar2=None,
                op0=_ALU.mult,
            )
            hc = spool.tile([128, 128], _F32, tag="hc")
            nc.vector.tensor_scalar(
                out=hc[:], in0=h_ps[:], scalar1=mu[:], scalar2=None,
                op0=_ALU.subtract,
            )
            sq = spool.tile([128, 128], _F32, tag="sq")
            ssq = spool.tile([128, 1], _F32, tag="ssq")
            nc.vector.scalar_tensor_tensor(
                out=sq[:], in0=hc[:], scalar=1.0, in1=hc[:],
                op0=_ALU.mult, op1=_ALU.mult, accum_out=ssq[:],
            )
            std = spool.tile([128, 1], _F32, tag="std")
            nc.scalar.activation(
                std[:], ssq[:], _ACT.Sqrt,
                bias=epsg_t[:, m : m + 1], scale=1.0 / 128,
            )
            rstd = spool.tile([128, 1], _F32, tag="rstd")
            nc.vector.reciprocal(rstd[:], std[:])
            y = spool.tile([128, 128], _F32, tag="y")
            nc.vector.scalar_tensor_tensor(
                out=y[:], in0=hc[:], scalar=rstd[:], in1=bc[:, 0:128],
                op0=_ALU.mult, op1=_ALU.mult,
            )
            y2 = spool.tile([128, 128], _F32, tag="y2")
            nc.vector.tensor_tensor(
                out=y2[:], in0=y[:], in1=bc[:, 128:256], op=_ALU.add
            )
            yl = spool.tile([128, 128], _F32, tag="yl")
            nc.vector.scalar_tensor_tensor(
                out=yl[:], in0=y2[:], scalar=NEG_SLOPE, in1=y2[:],
                op0=_ALU.mult, op1=_ALU.max,
            )
            prod = spool.tile([128, 128], _F32, tag="prod")
            oc = spool.tile([128, 1], _F32, tag="oc")
            nc.vector.scalar_tensor_tensor(
                out=prod[:], in0=yl[:], scalar=1.0, in1=bc[:, 256:384],
                op0=_ALU.mult, op1=_ALU.mult, accum_out=oc[:],
            )
            ofin = spool.tile([128, 1], _F32, tag="ofin")
            nc.vector.tensor_scalar(
                out=ofin[:], in0=oc[:], scalar1=bc[:, 384:385], scalar2=None,
                op0=_ALU.add,
            )
            nc.tensor.matmul(
                out=wps[0:1, 0:128], lhsT=ofin[:], rhs=id128_t,
                start=True, stop=True,
            )
            orow = spool.tile([1, 128], _F32, tag="orows")
            nc.scalar.copy(orow[:], wps[0:1, 0:128])
            nc.sync.dma_start(out=out_d[m, :], in_=orow[:])

        # ---- main stream ----
        schedule = {}
        for m in (0, 1):
            cA = -((-(4 * (m + 1)) * G) // K2) - 1  # chunk closing block 4m+3
            schedule.setdefault(cA, []).append(lambda m=m: head_stage_a(m))
        cB0 = -((-4 * G) // K2) + 1
        schedule.setdefault(cB0, []).append(lambda: head_stage_b(0))
        tail_fns = [lambda: head_stage_c(0), lambda: head_stage_b(1),
                    lambda: head_stage_c(1)]
        for c in range(NCH):
            ft = fpool.tile([128, K2, 2, 256], _F8, tag="ft")
            dma_eng = (nc.sync, nc.scalar)[c % 2]
            dma_eng.dma_start(
                out=ft[:],
                in_=feat_d[c].rearrange("p (k i f) -> p k i f", k=K2, i=2),
            )
            oh = opool.tile([128, 2 * K2, W], _F8, tag="oh")
            nc.vector.tensor_tensor(
                out=oh[:],
                in0=iota_t[:],
                in1=segT_t[:, 2 * K2 * c : 2 * K2 * (c + 1)].unsqueeze(2).broadcast_to(
                    [128, 2 * K2, W]),
                op=_ALU.is_equal,
            )
            for k in range(K2):
                g = K2 * c + k
                b = g // G
                nc.tensor.matmul(
                    out=accs[b],
                    lhsT=oh[:, 2 * k : 2 * k + 2, :],
                    rhs=ft[:, k, :, :],
                    start=(g % G == 0),
                    stop=(g % G == G - 1),
                    perf_mode=mybir.MatmulPerfMode.DoubleRow,
                )
            for fn in schedule.pop(c, []):
                fn()
        for c in sorted(schedule):
            for fn in schedule[c]:
                fn()
        for fn in tail_fns:
            fn()

    _split_excess_waits(nc)
    return nc


def _ef_quantize(feats: np.ndarray) -> np.ndarray:
    """fp8(e4m3) quantization with error feedback along the node axis.

    Carry chains run over consecutive nodes (cut every 128 for
    vectorization); since batch ids are sorted, per-segment sums of the
    quantized stream telescope to the exact sum plus O(1 ulp) carry terms.
    """
    n, d = feats.shape
    nchain = (n + 127) // 128
    pad = nchain * 128 - n
    x = np.concatenate([feats, np.zeros((pad, d), np.float32)], axis=0) if pad else feats
    x = x.reshape(nchain, 128, d)
    q = np.empty((nchain, 128, d), _FP8NP)
    c = np.zeros((nchain, d), np.float32)
    for p in range(128):
        y = x[:, p, :] + c
        qp = y.astype(_FP8NP)
        q[:, p, :] = qp
        c = y - qp.astype(np.float32)
    return q.reshape(nchain * 128, d)[:n]


def _prep_inputs(features, batch):
    feats = np.ascontiguousarray(np.asarray(features), dtype=np.float32)
    seg = np.asarray(batch).astype(np.int64)
    n = seg.shape[0]
    counts = np.bincount(seg, minlength=NUM_GRAPHS)
    bnd = np.zeros(NUM_GRAPHS + 1, np.int64)
    bnd[1:] = np.cumsum(counts)
    blk_lo = bnd[0:NUM_GRAPHS:W]  # [64]
    blk_hi = bnd[W : NUM_GRAPHS + 1 : W]
    blk_n = blk_hi - blk_lo
    G = int(max(32, np.max((blk_n + 255) // 256)))
    if (8 * G) % K2 != 0:
        G += (K2 - (8 * G) % K2 + 7) // 8  # bump G so 8G is a K2 multiple

    q = _ef_quantize(feats)  # [n, 256] fp8

    # node (core, g, ii, p) -> global node id, valid mask
    core = np.arange(N_CORES)[:, None, None, None]
    g = np.arange(NBLK * G)[None, :, None, None]
    ii = np.arange(2)[None, None, :, None]
    p = np.arange(128)[None, None, None, :]
    b = g // G
    gb = NBLK * core + b  # global block id [8, 8G, 1, 1]
    off = (g - b * G) * 256 + ii * 128 + p
    node = blk_lo[gb] + off
    valid = node < blk_hi[gb]
    node_c = np.minimum(node, n - 1)

    feat_dev = q[node_c]  # [8, 8G, 2, 128, 256] fp8
    feat_dev[~valid] = 0
    # device layout [core, chunk, p, k, ii, f]
    nch = 8 * G // K2
    feat_dev = feat_dev.reshape(N_CORES, nch, K2, 2, 128, 256).transpose(0, 1, 4, 2, 3, 5)
    feat_dev = np.ascontiguousarray(feat_dev).reshape(N_CORES, nch, 128, K2 * 2 * 256)

    segv = np.where(valid, seg[node_c] - W * gb, -1).astype(np.float32)  # [8, 8G, 2, 128]
    # segT[core, p, 2g+ii]
    segT = np.ascontiguousarray(segv.transpose(0, 3, 1, 2)).reshape(N_CORES, 128, -1)
    segT = segT.astype(ml_dtypes.bfloat16)
    return feat_dev, segT, counts, G


def kernel(features, batch, W1, b1, gamma, beta, W2, b2):
    feat_dev, segT, counts, G = _prep_inputs(features, batch)

    iota = np.tile(
        np.arange(W, dtype=np.float32)[None, None, :], (128, 2 * K2, 1)
    ).astype(ml_dtypes.bfloat16)

    blob = np.zeros((N_CORES, 128, 1187), np.float32)
    w1 = np.asarray(W1, np.float32).reshape(2, 128, 128).transpose(1, 0, 2)
    blob[:, :, 0:256] = w1.reshape(128, 256)[None]
    blob[:, :, 256:384] = np.eye(128, dtype=np.float32)[None]
    blob[:, 0:32, 384:416] = np.eye(32, dtype=np.float32)[None]
    epsg = (
        LN_EPS * np.maximum(counts.astype(np.float32), 1.0) ** 2
    ).reshape(N_CORES, 2, 128).transpose(0, 2, 1)
    blob[:, :, 416:418] = epsg
    blob[:, 0, 418:546] = np.asarray(b1, np.float32).ravel()[None]
    blob[:, 0, 546:802] = counts.astype(np.float32).reshape(N_CORES, 256)
    blob[:, 0, 802:1187] = np.concatenate(
        [
            np.asarray(gamma, np.float32).ravel(),
            np.asarray(beta, np.float32).ravel(),
            np.asarray(W2, np.float32).ravel(),
            np.asarray(b2, np.float32).ravel(),
        ]
    )[None]

    if G not in _PROGRAM_CACHE:
        _PROGRAM_CACHE[G] = _build_program(G)
    nc = _PROGRAM_CACHE[G]
    in_maps = [
        {
            "feat": feat_dev[i],
            "segT": segT[i],
            "iota": iota,
            "blob": blob[i],
        }
        for i in range(N_CORES)
    ]
    res = run_bass_kernel_spmd(
        nc, in_maps, list(range(N_CORES)), trace=PROFILE, tmpdir=PROFILE_DIR
    )
    global LAST_RESULT
    LAST_RESULT = res
    out = np.concatenate(
        [res.results[i]["out"].reshape(SEGS_PER_CORE) for i in range(N_CORES)]
    )
    return out.reshape(NUM_GRAPHS, 1).astype(np.float32)
